# revision 50
# baseline (speedup 1.0000x reference)
"""CSAB (cross-set attention block) Trainium2 kernel.

Full inputs in, full outputs out. Internally: data-parallel over batch
B=8 across the 8 NeuronCores (attention and projections are
batch-independent), one batch element per core.

Per-core dataflow (matmuls bf16/fp8, fp32 PSUM accumulation):
  - activations kept feature-major (transposed) so every matmul
    contracts over the partition dim with no on-chip transposes of the
    big score tensor:
      Q^T, K^T  [D, N]   from  lhsT=W chunks,  rhs=X^T
      V          [N, D]   token-major (lhsT=X^T chunk, rhs=W chunk),
                          fp8, packed per head-pair into a 192-col region
                          [V_A | 1 | 0*62 | 1 | V_B]; head A uses cols
                          0:128 of the region as lhsT, head B cols
                          64:192.  The extra output rows are free (matmul
                          cost is stream-bound), so each AV matmul yields
                          o' on one 64-row half and the softmax
                          denominator row riding along:
                            head A: rows 0:64 = o', row 64 = d
                            head B: row 0 = d, rows 64:128 = o'
      S^T[k,q]  = (K_h^T chunk).T @ Q_h^T   -- two heads of a pair run as
                  concurrent row-tiled matmuls (partitions 0:64 / 64:128)
      E^T       = exp(S^T / sqrt(D))  (ScalarE, scale folded in, fp8 out;
                  no max-subtraction needed: |S|/sqrt(D) < ~1)
      o'        = V'_h.T @ E_h^T  as fp8 DoubleRow matmuls (2 k-tiles of
                  128 per instruction)
      evict     [128,512] PSUM -> SBUF bf16 copy per head (one DVE op
                  carries o' and d together)
      d rows are DMA-scattered into a per-branch [32,256] tile; ONE wide
      reciprocal per branch (the per-row [1,512] reciprocal was 3.3us
      each -- free-dim serial); r is partition-broadcast by GPSIMD
      (no DRAM bounce), then o'*r + q^T (two DVE bf16 ops per head).
      fc: out^T accumulates W.T @ attn^T (attn includes the q-residual),
      then bias+relu+X residual.
"""

import math

import numpy as np
import ml_dtypes

import concourse.bass as bass
import concourse.mybir as mybir
import concourse.tile as tile
from concourse.bass_utils import run_bass_kernel_spmd

B, N, D, H = 8, 1024, 512, 8
DH = D // H          # 64
P = 128
KC = D // P          # 4 feature chunks
QH = N // 512        # 2 q halves
KT = N // P          # 8 k tiles
NPAIR = H // 2       # 4 head pairs
VW = 192             # per-pair V'aug region width: [V_A |1| 0*62 |1| V_B]
SCALE = 1.0 / math.sqrt(D)

F32 = mybir.dt.float32
BF16 = mybir.dt.bfloat16
FP8 = mybir.dt.float8e4
AF = mybir.ActivationFunctionType
ALU = mybir.AluOpType

USE_AV_DOUBLEROW = True
USE_PARTITION_BCAST = False

_BRANCHES = [("xx", "x", "x"), ("xy", "x", "y"), ("yx", "y", "x"), ("yy", "y", "y")]

LAST_RESULT = None
_CACHED_NC = None


def _split_excess_waits(nc):
    """The walrus build in this container accepts at most one sync-wait
    per instruction (two for EventSemaphore). Tile's scheduler emits
    several on some instructions. Hoist the overflow onto same-engine
    NoOps inserted immediately before the instruction — the engine
    blocks at the nops instead, so the wait point in the instruction
    stream is unchanged."""
    cap_of = {"InstEventSemaphore": 2}
    plans = []
    for f in nc.m.functions:
        for bb in f.blocks:
            plan = []
            for inst in list(bb.instructions):
                si = getattr(inst, "sync_info", None)
                waits = list(si.on_wait) if si and si.on_wait else []
                cap = cap_of.get(type(inst).__name__, 1)
                if len(waits) > cap:
                    plan.append((inst, waits[:-cap]))
                    si.on_wait = waits[-cap:]
            plans.append((bb, plan))
    nop_map = {}
    created = set()
    for bb, plan in plans:
        for inst, extra in plan:
            nops = []
            for w in extra:
                ni = nc.engines[inst.engine].nop(hint="waitsplit")
                ni.ins.sync_info = mybir.SyncInfo(on_wait=[w], on_update=[])
                nops.append(ni.ins)
                created.add(ni.ins.name)
            nop_map[inst.name] = nops
    for bb, plan in plans:
        live = [i for i in bb.instructions if i.name not in created]
        new = []
        for inst in live:
            new.extend(nop_map.get(inst.name, ()))
            new.append(inst)
        bb.instructions = new


def _build_nc():
    nc = bass.Bass()

    # ---- DRAM I/O (per core) -------------------------------------------
    xt_bf = nc.dram_tensor("xt_bf", [D, N], BF16, kind="ExternalInput")
    yt_bf = nc.dram_tensor("yt_bf", [D, N], BF16, kind="ExternalInput")
    wdr = {}
    for bn, _, _ in _BRANCHES:
        for t in ("q", "k", "v"):
            wdr[f"w{t}_{bn}"] = nc.dram_tensor(f"w{t}_{bn}", [D, D], BF16,
                                               kind="ExternalInput")
            wdr[f"b{t}_{bn}"] = nc.dram_tensor(f"b{t}_{bn}", [D], F32,
                                               kind="ExternalInput")
    wfc_x = nc.dram_tensor("wfc_x", [2 * D, D], BF16, kind="ExternalInput")
    wfc_y = nc.dram_tensor("wfc_y", [2 * D, D], BF16, kind="ExternalInput")
    bfc_x = nc.dram_tensor("bfc_x", [D], BF16, kind="ExternalInput")
    bfc_y = nc.dram_tensor("bfc_y", [D], BF16, kind="ExternalInput")
    out_x_t = nc.dram_tensor("out_x_t", [D, N], F32, kind="ExternalOutput")
    out_y_t = nc.dram_tensor("out_y_t", [D, N], F32, kind="ExternalOutput")

    with tile.TileContext(nc) as tc, \
         tc.tile_pool(name="const", bufs=1) as const_pool, \
         tc.tile_pool(name="wfc", bufs=1) as wfc_pool, \
         tc.tile_pool(name="acts", bufs=1) as acts_pool, \
         tc.tile_pool(name="wbr", bufs=2) as wbr_pool, \
         tc.tile_pool(name="qkv_q", bufs=2) as q_pool, \
         tc.tile_pool(name="qkv_kv", bufs=2) as kv_pool, \
         tc.tile_pool(name="epool", bufs=2) as e_pool, \
         tc.tile_pool(name="o2", bufs=18) as o2_pool, \
         tc.tile_pool(name="rbc", bufs=2) as rbc_pool, \
         tc.tile_pool(name="ntmp", bufs=2) as ntmp_pool, \
         tc.tile_pool(name="poly", bufs=2) as poly_pool, \
         tc.tile_pool(name="dgr", bufs=10) as dgr_pool, \
         tc.tile_pool(name="attn", bufs=3) as attn_pool, \
         tc.tile_pool(name="stream", bufs=2) as stream_pool, \
         tc.tile_pool(name="dbounce", bufs=4, space="DRAM") as dram_pool, \
         tc.tile_pool(name="qk_ps", bufs=1, space="PSUM") as qk_ps_pool, \
         tc.tile_pool(name="av_ps", bufs=2, space="PSUM") as av_ps_pool, \
         tc.tile_pool(name="proj_ps", bufs=2, space="PSUM") as proj_ps_pool:

        # ---- resident activations --------------------------------------
        # DMA order matters: the first projection (branch xx) needs xt
        # and wq_xx, so those go first; yt and the other branches'
        # weights stream behind them; wfc (first needed by fc_x, deep
        # into the kernel) goes last.
        xt_sb = acts_pool.tile([P, KC, N], BF16, tag="xt_sb")
        yt_sb = acts_pool.tile([P, KC, N], BF16, tag="yt_sb")
        for kc in range(KC):
            nc.sync.dma_start(
                xt_sb[:, kc, :],
                xt_bf.rearrange("(o p) n -> p o n", p=P)[:, kc, :])
        act_sb = {"x": xt_sb, "y": yt_sb}

        wsb_of = {}

        def load_branch_weights(bname):
            wl = {}
            for t in ("q", "k", "v"):
                w = wbr_pool.tile([P, KC, D], BF16, tag=f"w{t}")
                nc.sync.dma_start(
                    w[:], wdr[f"w{t}_{bname}"].rearrange("(o p) f -> p o f", p=P))
                wl[t] = w
            bq = wbr_pool.tile([P, KC], F32, tag="bq")
            nc.sync.dma_start(
                bq[:], wdr[f"bq_{bname}"].rearrange("(o p) -> p o", p=P))
            bk = wbr_pool.tile([P, KC], F32, tag="bk")
            nc.sync.dma_start(
                bk[:], wdr[f"bk_{bname}"].rearrange("(o p) -> p o", p=P))
            # V bias broadcast across all 128 partitions (tokens)
            bvb = wbr_pool.tile([P, D], F32, tag="bvb")
            nc.gpsimd.dma_start(
                out=bvb[:],
                in_=wdr[f"bv_{bname}"][:].partition_broadcast(P),
            )
            wl["bq"], wl["bk"], wl["bvb"] = bq, bk, bvb
            wsb_of[bname] = wl

        load_branch_weights("xx")
        for kc in range(KC):
            nc.sync.dma_start(
                yt_sb[:, kc, :],
                yt_bf.rearrange("(o p) n -> p o n", p=P)[:, kc, :])
        load_branch_weights("xy")

        wfc_sb = {}
        bfc_sb = {}

        ones_row = const_pool.tile([1, 512], BF16, tag="ones_row")
        nc.gpsimd.memset(ones_row[:], 1.0)

        def load_fc_weights(nm):
            # wfc_x and wfc_y share one ring slot: fc_x finishes (late
            # in yx's attention) before wfc_y is needed (the tail)
            wd, bd = (wfc_x, bfc_x) if nm == "x" else (wfc_y, bfc_y)
            w = wfc_pool.tile([P, 2 * KC, D], BF16, tag="wfc")
            nc.sync.dma_start(w[:], wd.rearrange("(o p) f -> p o f", p=P))
            bt = const_pool.tile([1, D], BF16, tag=f"bfc_{nm}")
            nc.sync.dma_start(bt[:], bd.rearrange("(a d) -> a d", a=1))
            wfc_sb[nm] = w
            bfc_sb[nm] = bt

        def proj_feature_major(src_sb, w_sb, b_sb, out_sb, ofc, qh):
            # out^T[of, q] = sum_kc W[kc, of].T @ src^T[kc, q]  (+ bias row)
            ps = proj_ps_pool.tile([P, 512], F32, tag="proj")
            for kc in range(KC):
                nc.tensor.matmul(
                    ps[:],
                    lhsT=w_sb[:, kc, ofc * P:(ofc + 1) * P],
                    rhs=src_sb[:, kc, qh * 512:(qh + 1) * 512],
                    start=(kc == 0), stop=(kc == KC - 1),
                )
            nc.vector.tensor_scalar(
                out_sb[:, ofc, qh * 512:(qh + 1) * 512],
                ps[:],
                b_sb[:, ofc:ofc + 1],
                None,
                ALU.add,
            )

        def proj_token_major_vaug(src_sb, w_sb, bvb_sb, vaug_sb, tt):
            # V[t, f] token-major fp8, scattered into per-pair 192-col
            # regions: head A at cols 0:64, head B at cols 128:192.
            ps = proj_ps_pool.tile([P, 512], F32, tag="proj")
            for kc in range(KC):
                nc.tensor.matmul(
                    ps[:],
                    lhsT=src_sb[:, kc, tt * P:(tt + 1) * P],
                    rhs=w_sb[:, kc, :],
                    start=(kc == 0), stop=(kc == KC - 1),
                )
            psv = ps[:].rearrange("p (pr s c) -> p pr s c", s=2, c=DH)
            bvv = bvb_sb[:].rearrange("p (pr s c) -> p pr s c", s=2, c=DH)
            nc.vector.tensor_tensor(
                vaug_sb[:, tt, :, 0:DH],
                psv[:, :, 0, :], bvv[:, :, 0, :], ALU.add,
            )
            nc.vector.tensor_tensor(
                vaug_sb[:, tt, :, 2 * DH:3 * DH],
                psv[:, :, 1, :], bvv[:, :, 1, :], ALU.add,
            )

        attn_sb_of = {}   # branch name -> attn^T (incl Q residual), bf16

        def fc_tasks(out_name, br0, br1):
            # one closure per (ofc, qh) output tile of the fc
            out_dram = out_x_t if out_name == "x" else out_y_t
            res_sb = act_sb[out_name]

            def one(ofc, qh):
                w_sb = wfc_sb[out_name]
                b_sb = bfc_sb[out_name]
                srcs = [(attn_sb_of[br0], 0), (attn_sb_of[br1], KC)]
                ps = proj_ps_pool.tile([P, 512], F32, tag="proj")
                step = 0
                for src_sb, wbase in srcs:
                    for kc in range(KC):
                        nc.tensor.matmul(
                            ps[:],
                            lhsT=w_sb[:, wbase + kc, ofc * P:(ofc + 1) * P],
                            rhs=src_sb[:, kc, qh * 512:(qh + 1) * 512],
                            start=(step == 0), stop=False,
                        )
                        step += 1
                # fc bias as a rank-1 matmul (contract dim 1 of ones)
                nc.tensor.matmul(
                    ps[:],
                    lhsT=b_sb[0:1, ofc * P:(ofc + 1) * P],
                    rhs=ones_row[0:1, :],
                    start=False, stop=True,
                )
                # epilogue: relu(fc) + residual in one fused op; the
                # residual comes from the resident bf16 activations
                outt = stream_pool.tile([P, 512], F32, tag="outt")
                nc.vector.scalar_tensor_tensor(
                    outt[:], ps[:], 0.0,
                    res_sb[:, ofc, qh * 512:(qh + 1) * 512],
                    ALU.max, ALU.add)
                nc.sync.dma_start(
                    out_dram[ofc * P:(ofc + 1) * P, qh * 512:(qh + 1) * 512],
                    outt[:],
                )

            return [(lambda ofc=ofc, qh=qh: one(ofc, qh))
                    for ofc in range(KC) for qh in range(QH)]

        def fc_split(out_name, br0, br1, tiles, pool):
            # phase A: the br0-side contraction + bias (ready before
            # br1's normalize); returns phase-B completions
            out_dram = out_x_t if out_name == "x" else out_y_t
            res_sb = act_sb[out_name]
            w_sb = wfc_sb[out_name]
            b_sb = bfc_sb[out_name]
            done = []
            for ofc, qh in tiles:
                ps = pool.tile([P, 512], F32, tag="proj" if pool is proj_ps_pool else "av")
                src0 = attn_sb_of[br0]
                for kc in range(KC):
                    nc.tensor.matmul(
                        ps[:],
                        lhsT=w_sb[:, kc, ofc * P:(ofc + 1) * P],
                        rhs=src0[:, kc, qh * 512:(qh + 1) * 512],
                        start=(kc == 0), stop=False,
                    )
                nc.tensor.matmul(
                    ps[:],
                    lhsT=b_sb[0:1, ofc * P:(ofc + 1) * P],
                    rhs=ones_row[0:1, :],
                    start=False, stop=False,
                )

                def fin(ofc=ofc, qh=qh, ps=ps):
                    src1 = attn_sb_of[br1]
                    for kc in range(KC):
                        nc.tensor.matmul(
                            ps[:],
                            lhsT=w_sb[:, KC + kc, ofc * P:(ofc + 1) * P],
                            rhs=src1[:, kc, qh * 512:(qh + 1) * 512],
                            start=False, stop=(kc == KC - 1),
                        )
                    outt = stream_pool.tile([P, 512], F32, tag="outt")
                    nc.vector.scalar_tensor_tensor(
                        outt[:], ps[:], 0.0,
                        res_sb[:, ofc, qh * 512:(qh + 1) * 512],
                        ALU.max, ALU.add)
                    nc.sync.dma_start(
                        out_dram[ofc * P:(ofc + 1) * P,
                                 qh * 512:(qh + 1) * 512],
                        outt[:],
                    )
                done.append(fin)
            return done

        # ---- per-branch state + emission helpers -------------------------
        class Branch:
            pass

        def start_branch(bname, qsrc, kvsrc):
            """Emit weight DMAs + vaug memsets; return Branch with
            per-tile proj closures (to be interleaved as PE filler)."""
            br = Branch()
            br.name = bname
            br.quarters = (bname == "yy")
            w_sb = wsb_of[bname]
            bq_sb = w_sb["bq"]
            bk_sb = w_sb["bk"]
            bvb_sb = w_sb["bvb"]
            br.qt = q_pool.tile([P, KC, N], BF16, tag="qt")
            br.kt = kv_pool.tile([P, KC, N], BF16, tag="kt")
            br.vaug = kv_pool.tile([P, KT, NPAIR, VW], FP8, tag="vaug")
            # ones columns at region cols 64 and 127; zeros between
            nc.gpsimd.memset(br.vaug[:, :, :, DH:DH + 1], 1.0)
            nc.gpsimd.memset(br.vaug[:, :, :, DH + 1:2 * DH - 1], 0.0)
            nc.gpsimd.memset(br.vaug[:, :, :, 2 * DH - 1:2 * DH], 1.0)

            qs, kvs = act_sb[qsrc], act_sb[kvsrc]
            pre, early, rest = [], [], []
            for ofc in range(KC):
                for qh in range(QH):
                    # S of pair p reads BOTH K qh-halves of chunk p (the
                    # k axis spans all of N) but only its own Q half, so
                    # K(0,1) must precede the first S while Q(0,1) and V
                    # can ride in the first attention slot.
                    qdst = pre if (ofc == 0 and qh == 0) else (
                        early if ofc == 0 else rest)
                    kdst = pre if ofc == 0 else rest
                    qdst.append(lambda ofc=ofc, qh=qh: proj_feature_major(
                        qs, w_sb["q"], bq_sb, br.qt, ofc, qh))
                    kdst.append(lambda ofc=ofc, qh=qh: proj_feature_major(
                        kvs, w_sb["k"], bk_sb, br.kt, ofc, qh))
            for tt in range(KT):
                early.append(lambda tt=tt: proj_token_major_vaug(
                    kvs, w_sb["v"], bvb_sb, br.vaug, tt))
            br.pre_tasks = pre
            br.early_tasks = early
            br.rest_tasks = rest
            br.proj_tasks = pre + early + rest
            return br

        def emit_s_block(br, pair, qh, t2, e_sb):
            # 2 k-tiles x 2 heads into one [128,2048] PSUM tile (4
            # banks), ONE exp over all 2048 columns: halves the ACT
            # instruction count.  The single-buffer qk pool means the
            # next block's matmuls wait for this exp, so callers
            # interleave other PE work between blocks.
            ps = qk_ps_pool.tile([P, 2048], F32, tag="qk")
            for j in range(2):
                kt = 2 * t2 + j
                nc.tensor.matmul(
                    ps[:, j * 1024:j * 1024 + 512],
                    lhsT=br.kt[0:DH, pair, kt * P:(kt + 1) * P],
                    rhs=br.qt[0:DH, pair, qh * 512:(qh + 1) * 512],
                    start=True, stop=True,
                )
                nc.tensor.matmul(
                    ps[:, j * 1024 + 512:j * 1024 + 1024],
                    lhsT=br.kt[DH:P, pair, kt * P:(kt + 1) * P],
                    rhs=br.qt[DH:P, pair, qh * 512:(qh + 1) * 512],
                    start=True, stop=True,
                )
            nc.scalar.activation(
                e_sb[:, 2 * t2:2 * t2 + 2, :, :], ps[:],
                AF.Exp, scale=SCALE,
            )

        def emit_s_exp(br, pair, qh, work=()):
            # `work` closures are injected between the 4 S/exp blocks
            # to keep the PE fed while the single qk buffer drains
            e_sb = e_pool.tile([P, KT, 2, 512], FP8, tag="e")
            work = list(work)
            nw = len(work)
            wi = 0
            for t2 in range(4):
                emit_s_block(br, pair, qh, t2, e_sb)
                upto = (t2 + 1) * nw // 4
                while wi < upto:
                    work[wi]()
                    wi += 1
            return e_sb

        def pe_warm(pool, n):
            # tiny matmuls into a scratch PSUM tile: keep the PE busy
            # through otherwise-idle stretches so HAM stays at K=8/8
            # (a >3.4us PE lull re-throttles the clock to 1.2 GHz)
            tag = "proj" if pool is proj_ps_pool else "av"
            for _ in range(n):
                scr = pool.tile([P, 512], F32, tag=tag)
                nc.tensor.matmul(
                    scr[0:1, :], lhsT=ones_row[0:1, 0:1], rhs=ones_row[0:1, :],
                    start=True, stop=True)

        def dg_loc(br, pair, qh, hl):
            if br.quarters:
                return pair, 4 * (qh * 2 + hl)
            return pair // 2, 4 * ((pair % 2) * 4 + qh * 2 + hl)

        def emit_av(br, pair, qh, e_sb):
            for hl in range(2):
                emit_av_head(br, pair, qh, hl, e_sb)

        def emit_av_head(br, pair, qh, hl, e_sb):
            if True:
                ps_av = av_ps_pool.tile([P, 512], F32, tag="av")
                if USE_AV_DOUBLEROW:
                    # kt-pair (0,1) last: its E may come from the DVE
                    # polynomial, which finishes later than ACT exps
                    for t2 in (1, 2, 3, 0):
                        nc.tensor.matmul(
                            ps_av[:],
                            lhsT=br.vaug[:, 2 * t2:2 * t2 + 2, pair,
                                         hl * DH:hl * DH + P],
                            rhs=e_sb[:, 2 * t2:2 * t2 + 2, hl, :],
                            start=(t2 == 1), stop=(t2 == 0),
                            perf_mode=mybir.MatmulPerfMode.DoubleRow,
                        )
                else:
                    for kt in range(KT):
                        nc.tensor.matmul(
                            ps_av[:],
                            lhsT=br.vaug[:, kt, pair, hl * DH:hl * DH + P],
                            rhs=e_sb[:, kt, hl, :],
                            start=(kt == 0), stop=(kt == KT - 1),
                        )
                o2 = o2_pool.tile([P, 512], BF16, tag="o2")
                nc.vector.tensor_copy(o2[:], ps_av[:])
                br.o2_tiles[(pair, qh, hl)] = o2
                # scatter d row into the branch dg tile as [2, 256]
                drow = DH if hl == 0 else 0
                tidx, row = dg_loc(br, pair, qh, hl)
                nc.sync.dma_start(
                    br.dg[tidx][row:row + 4, :],
                    o2[drow:drow + 1, :],
                )

        def attention_branch(br, filler, late_filler=(), early_filler=(),
                             pend=None):
            """Emit the branch's attention with software pipelining:
            the AV of one group is emitted after the S+exp of the NEXT
            group (even across branch boundaries, via `pend`), and
            `filler` closures (next branch's projections, the previous
            branch's normalize, ready fc tiles) are interleaved to keep
            the PE fed during the ACT-bound exp stretches."""
            br.o2_tiles = {}
            attn_t = attn_pool.tile([P, KC, N], BF16, tag="attn")
            br.attn = attn_t
            attn_sb_of[br.name] = attn_t
            nt = 4 if br.quarters else 2
            rows = 64 // nt
            br.dg = []
            br.rr = []
            for h in range(nt):
                dg_t = dgr_pool.tile([rows, 128], BF16, tag=f"dg{h}")
                rr_t = dgr_pool.tile([rows, 128], BF16, tag=f"rr{h}")
                br.dg.append(dg_t)
                br.rr.append(rr_t)
            groups = [(pair, qh) for pair in range(NPAIR)
                      for qh in range(QH)][getattr(br, "skip_groups", 0):]
            nslots = len(groups) + 1
            fill_plan = [[] for _ in range(nslots)]
            for i, task in enumerate(filler):
                fill_plan[min(i * nslots // max(len(filler), 1), nslots - 1)].append(task)
            fill_plan[0][0:0] = list(early_filler)
            fill_plan[6].extend(late_filler)
            for g, (pair, qh) in enumerate(groups):
                work = []
                if pend is not None:
                    pbr, ppair, pqh, pe_sb = pend
                    work.append(lambda: emit_av_head(pbr, ppair, pqh, 0, pe_sb))
                    work.append(lambda: emit_av_head(pbr, ppair, pqh, 1, pe_sb))
                work.extend(fill_plan[g])
                e_sb = emit_s_exp(br, pair, qh, work=work)
                pend = (br, pair, qh, e_sb)
            for task in fill_plan[len(groups)]:
                task()
            return pend

        def normalize_tile(br, tidx, pairs):
            # one wide reciprocal over the dg tile, then normalize its
            # pairs
            with nc.allow_low_precision("softmax denominators in bf16"):
                nc.vector.reciprocal(br.rr[tidx][:], br.dg[tidx][:])
            nrows = br.dg[tidx].shape[0]
            r_dram = dram_pool.tile([nrows, 128], BF16, tag="rd")
            nc.sync.dma_start(r_dram[:], br.rr[tidx][:])
            r_flat = r_dram.rearrange("a b -> (a b)")
            attn_sb = br.attn
            for pair in pairs:
                for qh in range(QH):
                    rbc = rbc_pool.tile([P, 512], BF16, tag="rbc")
                    tmp = ntmp_pool.tile([P, 512], BF16, tag="ntmp")
                    for hl in range(2):
                        rows = slice(hl * DH, (hl + 1) * DH)
                        hidx = dg_loc(br, pair, qh, hl)[1] // 4
                        nc.gpsimd.dma_start(
                            out=rbc[rows, :],
                            in_=r_flat[hidx * 512:(hidx + 1) * 512]
                            .partition_broadcast(DH),
                        )
                        o2 = br.o2_tiles[(pair, qh, hl)]
                        orows = slice(0, DH) if hl == 0 else slice(DH, P)
                        nc.vector.tensor_tensor(
                            tmp[rows, :], o2[orows, :], rbc[rows, :], ALU.mult)
                        nc.vector.tensor_tensor(
                            attn_sb[rows, pair, qh * 512:(qh + 1) * 512],
                            tmp[rows, :],
                            br.qt[rows, pair, qh * 512:(qh + 1) * 512],
                            ALU.add,
                        )

        def normalize_half(br, half):
            normalize_tile(br, half, (2 * half, 2 * half + 1))

        def normalize_quarter(br, pair):
            normalize_tile(br, pair, (pair,))

        def normalize_branch(br):
            normalize_half(br, 0)
            normalize_half(br, 1)

        # ---- pipelined branch schedule ----------------------------------
        # warm the ACT table (exp set) at t=0 with a dummy activation
        warm = const_pool.tile([1, 512], F32, tag="warm")
        nc.scalar.activation(warm[:], ones_row[:], AF.Exp, scale=1.0)

        br_xx = start_branch(*_BRANCHES[0])
        # handcrafted prologue: Q(0,0) + K(0,0) projections, then the
        # first 4 S k-tiles (which only read K columns 0:512), with
        # K(0,1) landing while their exps run
        for t in br_xx.pre_tasks:
            t()
        br_xy = start_branch(*_BRANCHES[1])
        pend = attention_branch(
            br_xx,
            [lambda: load_branch_weights("yx"),
             lambda: load_fc_weights("x")]
            + br_xx.rest_tasks + br_xy.proj_tasks,
            early_filler=br_xx.early_tasks)
        br_yx = start_branch(*_BRANCHES[2])
        pend = attention_branch(
            br_xy,
            [lambda: normalize_branch(br_xx),
             lambda: load_branch_weights("yy")]
            + br_yx.proj_tasks,
            pend=pend)
        fcx = fc_tasks("x", "xx", "xy")
        br_yy = start_branch(*_BRANCHES[3])
        pend = attention_branch(
            br_yx,
            [lambda: normalize_branch(br_xy)] + br_yy.proj_tasks + fcx[0:6],
            pend=pend)
        finish = []
        pend = attention_branch(
            br_yy,
            [lambda: normalize_branch(br_yx),
             lambda: load_fc_weights("y")],
            late_filler=[
                lambda: normalize_quarter(br_yy, 0),
                lambda: normalize_quarter(br_yy, 1),
                lambda: finish.extend(
                    fc_split("y", "yx", "yy", [(0, 0), (0, 1)],
                             proj_ps_pool)),
            ],
            early_filler=fcx[6:8],
            pend=pend)
        emit_av(pend[0], *pend[1:])
        normalize_quarter(br_yy, 2)
        fcy = fc_tasks("y", "yx", "yy")
        finish += fc_split("y", "yx", "yy", [(1, 0), (1, 1)], av_ps_pool)
        normalize_quarter(br_yy, 3)
        for f in finish:
            f()
        for t in fcy[4:8]:
            t()

    _split_excess_waits(nc)
    return nc


def _get_nc():
    global _CACHED_NC
    if _CACHED_NC is None:
        _CACHED_NC = _build_nc()
    return _CACHED_NC


def kernel(**inputs):
    global LAST_RESULT
    nc = _get_nc()

    X = np.asarray(inputs["X"], np.float32)
    Y = np.asarray(inputs["Y"], np.float32)

    def bf(a):
        return np.ascontiguousarray(a).astype(ml_dtypes.bfloat16)

    shared = {}
    for bn, _, _ in _BRANCHES:
        for t in ("q", "k", "v"):
            shared[f"w{t}_{bn}"] = bf(inputs[f"W_{t}_{bn}"])
            shared[f"b{t}_{bn}"] = np.asarray(inputs[f"b_{t}_{bn}"], np.float32)
    shared["wfc_x"] = bf(inputs["W_X"])
    shared["wfc_y"] = bf(inputs["W_Y"])
    shared["bfc_x"] = bf(inputs["b_X"])
    shared["bfc_y"] = bf(inputs["b_Y"])

    in_maps = []
    for b in range(B):
        xt = np.ascontiguousarray(X[b].T)
        yt = np.ascontiguousarray(Y[b].T)
        m = dict(shared)
        m["xt_bf"] = xt.astype(ml_dtypes.bfloat16)
        m["yt_bf"] = yt.astype(ml_dtypes.bfloat16)
        in_maps.append(m)

    res = run_bass_kernel_spmd(nc, in_maps, list(range(B)))
    LAST_RESULT = res

    out_x = np.stack([res.results[b]["out_x_t"].T for b in range(B)])
    out_y = np.stack([res.results[b]["out_y_t"].T for b in range(B)])
    return out_x.astype(np.float32), out_y.astype(np.float32)


# revision 51
# speedup vs baseline: 1.2703x; 1.2703x over previous
"""CSAB (cross-set attention block) Trainium2 kernel.

Full inputs in, full outputs out. Internally: data-parallel over batch
B=8 across the 8 NeuronCores (attention and projections are
batch-independent), one batch element per core.

Per-core dataflow (matmuls bf16/fp8, fp32 PSUM accumulation):
  - activations kept feature-major (transposed) so every matmul
    contracts over the partition dim with no on-chip transposes of the
    big score tensor:
      Q^T, K^T  [D, N]   from  lhsT=W chunks,  rhs=X^T
      V          [N, D]   token-major (lhsT=X^T chunk, rhs=W chunk),
                          fp8, packed per head-pair into a 192-col region
                          [V_A | 1 | 0*62 | 1 | V_B]; head A uses cols
                          0:128 of the region as lhsT, head B cols
                          64:192.  The extra output rows are free (matmul
                          cost is stream-bound), so each AV matmul yields
                          o' on one 64-row half and the softmax
                          denominator row riding along:
                            head A: rows 0:64 = o', row 64 = d
                            head B: row 0 = d, rows 64:128 = o'
      S^T[k,q]  = (K_h^T chunk).T @ Q_h^T   -- two heads of a pair run as
                  concurrent row-tiled matmuls (partitions 0:64 / 64:128)
      E^T       = exp(S^T / sqrt(D))  (ScalarE, scale folded in, fp8 out;
                  no max-subtraction needed: |S|/sqrt(D) < ~1)
      o'        = V'_h.T @ E_h^T  as fp8 DoubleRow matmuls (2 k-tiles of
                  128 per instruction)
      evict     [128,512] PSUM -> SBUF bf16 copy per head (one DVE op
                  carries o' and d together)
      d rows are DMA-scattered into a per-branch [32,256] tile; ONE wide
      reciprocal per branch (the per-row [1,512] reciprocal was 3.3us
      each -- free-dim serial); r is partition-broadcast by GPSIMD
      (no DRAM bounce), then o'*r + q^T (two DVE bf16 ops per head).
      fc: out^T accumulates W.T @ attn^T (attn includes the q-residual),
      then bias+relu+X residual.
"""

import math

import numpy as np
import ml_dtypes

import concourse.bass as bass
import concourse.mybir as mybir
import concourse.tile as tile
from concourse.bass_utils import run_bass_kernel_spmd

B, N, D, H = 8, 1024, 512, 8
DH = D // H          # 64
P = 128
KC = D // P          # 4 feature chunks
QH = N // 512        # 2 q halves
KT = N // P          # 8 k tiles
NPAIR = H // 2       # 4 head pairs
VW = 192             # per-pair V'aug region width: [V_A |1| 0*62 |1| V_B]
SCALE = 1.0 / math.sqrt(D)

F32 = mybir.dt.float32
BF16 = mybir.dt.bfloat16
FP8 = mybir.dt.float8e4
AF = mybir.ActivationFunctionType
ALU = mybir.AluOpType

USE_AV_DOUBLEROW = True
USE_PARTITION_BCAST = False

_BRANCHES = [("xx", "x", "x"), ("xy", "x", "y"), ("yx", "y", "x"), ("yy", "y", "y")]

LAST_RESULT = None
_CACHED_NC = None


def _split_excess_waits(nc):
    """The walrus build in this container accepts at most one sync-wait
    per instruction (two for EventSemaphore). Tile's scheduler emits
    several on some instructions. Hoist the overflow onto same-engine
    NoOps inserted immediately before the instruction — the engine
    blocks at the nops instead, so the wait point in the instruction
    stream is unchanged."""
    cap_of = {"InstEventSemaphore": 2}
    plans = []
    for f in nc.m.functions:
        for bb in f.blocks:
            plan = []
            for inst in list(bb.instructions):
                si = getattr(inst, "sync_info", None)
                waits = list(si.on_wait) if si and si.on_wait else []
                cap = cap_of.get(type(inst).__name__, 1)
                if len(waits) > cap:
                    plan.append((inst, waits[:-cap]))
                    si.on_wait = waits[-cap:]
            plans.append((bb, plan))
    nop_map = {}
    created = set()
    for bb, plan in plans:
        for inst, extra in plan:
            nops = []
            for w in extra:
                ni = nc.engines[inst.engine].nop(hint="waitsplit")
                ni.ins.sync_info = mybir.SyncInfo(on_wait=[w], on_update=[])
                nops.append(ni.ins)
                created.add(ni.ins.name)
            nop_map[inst.name] = nops
    for bb, plan in plans:
        live = [i for i in bb.instructions if i.name not in created]
        new = []
        for inst in live:
            new.extend(nop_map.get(inst.name, ()))
            new.append(inst)
        bb.instructions = new


def _build_nc():
    nc = bass.Bass()

    # ---- DRAM I/O (per core) -------------------------------------------
    xt_bf = nc.dram_tensor("xt_bf", [D, N], BF16, kind="ExternalInput")
    yt_bf = nc.dram_tensor("yt_bf", [D, N], BF16, kind="ExternalInput")
    wdr = {}
    for bn, _, _ in _BRANCHES:
        for t in ("q", "k", "v"):
            wdr[f"w{t}_{bn}"] = nc.dram_tensor(f"w{t}_{bn}", [D, D], BF16,
                                               kind="ExternalInput")
            wdr[f"b{t}_{bn}"] = nc.dram_tensor(f"b{t}_{bn}", [D], F32,
                                               kind="ExternalInput")
    wfc_x = nc.dram_tensor("wfc_x", [2 * D, D], BF16, kind="ExternalInput")
    wfc_y = nc.dram_tensor("wfc_y", [2 * D, D], BF16, kind="ExternalInput")
    bfc_x = nc.dram_tensor("bfc_x", [D], BF16, kind="ExternalInput")
    bfc_y = nc.dram_tensor("bfc_y", [D], BF16, kind="ExternalInput")
    out_x_t = nc.dram_tensor("out_x_t", [D, N], F32, kind="ExternalOutput")
    out_y_t = nc.dram_tensor("out_y_t", [D, N], F32, kind="ExternalOutput")

    with tile.TileContext(nc) as tc, \
         tc.tile_pool(name="const", bufs=1) as const_pool, \
         tc.tile_pool(name="wfc", bufs=1) as wfc_pool, \
         tc.tile_pool(name="acts", bufs=1) as acts_pool, \
         tc.tile_pool(name="wbr", bufs=2) as wbr_pool, \
         tc.tile_pool(name="qkv_q", bufs=2) as q_pool, \
         tc.tile_pool(name="qkv_kv", bufs=2) as kv_pool, \
         tc.tile_pool(name="epool", bufs=2) as e_pool, \
         tc.tile_pool(name="o2", bufs=18) as o2_pool, \
         tc.tile_pool(name="rbc", bufs=2) as rbc_pool, \
         tc.tile_pool(name="ntmp", bufs=2) as ntmp_pool, \
         tc.tile_pool(name="poly", bufs=2) as poly_pool, \
         tc.tile_pool(name="dgr", bufs=10) as dgr_pool, \
         tc.tile_pool(name="attn", bufs=3) as attn_pool, \
         tc.tile_pool(name="stream", bufs=2) as stream_pool, \
         tc.tile_pool(name="dbounce", bufs=4, space="DRAM") as dram_pool, \
         tc.tile_pool(name="qk_ps", bufs=2, space="PSUM") as qk_ps_pool, \
         tc.tile_pool(name="av_ps", bufs=2, space="PSUM") as av_ps_pool, \
         tc.tile_pool(name="proj_ps", bufs=2, space="PSUM") as proj_ps_pool:

        # ---- resident activations --------------------------------------
        # DMA order matters: the first projection (branch xx) needs xt
        # and wq_xx, so those go first; yt and the other branches'
        # weights stream behind them; wfc (first needed by fc_x, deep
        # into the kernel) goes last.
        xt_sb = acts_pool.tile([P, KC, N], BF16, tag="xt_sb")
        yt_sb = acts_pool.tile([P, KC, N], BF16, tag="yt_sb")
        for kc in range(KC):
            nc.sync.dma_start(
                xt_sb[:, kc, :],
                xt_bf.rearrange("(o p) n -> p o n", p=P)[:, kc, :])
        act_sb = {"x": xt_sb, "y": yt_sb}

        wsb_of = {}

        def load_branch_weights(bname):
            wl = {}
            for t in ("q", "k", "v"):
                w = wbr_pool.tile([P, KC, D], BF16, tag=f"w{t}")
                nc.sync.dma_start(
                    w[:], wdr[f"w{t}_{bname}"].rearrange("(o p) f -> p o f", p=P))
                wl[t] = w
            bq = wbr_pool.tile([P, KC], F32, tag="bq")
            nc.sync.dma_start(
                bq[:], wdr[f"bq_{bname}"].rearrange("(o p) -> p o", p=P))
            bk = wbr_pool.tile([P, KC], F32, tag="bk")
            nc.sync.dma_start(
                bk[:], wdr[f"bk_{bname}"].rearrange("(o p) -> p o", p=P))
            # V bias broadcast across all 128 partitions (tokens)
            bvb = wbr_pool.tile([P, D], F32, tag="bvb")
            nc.gpsimd.dma_start(
                out=bvb[:],
                in_=wdr[f"bv_{bname}"][:].partition_broadcast(P),
            )
            wl["bq"], wl["bk"], wl["bvb"] = bq, bk, bvb
            wsb_of[bname] = wl

        load_branch_weights("xx")
        for kc in range(KC):
            nc.sync.dma_start(
                yt_sb[:, kc, :],
                yt_bf.rearrange("(o p) n -> p o n", p=P)[:, kc, :])
        load_branch_weights("xy")

        wfc_sb = {}
        bfc_sb = {}

        ones_row = const_pool.tile([1, 512], BF16, tag="ones_row")
        nc.gpsimd.memset(ones_row[:], 1.0)

        def load_fc_weights(nm):
            # wfc_x and wfc_y share one ring slot: fc_x finishes (late
            # in yx's attention) before wfc_y is needed (the tail)
            wd, bd = (wfc_x, bfc_x) if nm == "x" else (wfc_y, bfc_y)
            w = wfc_pool.tile([P, 2 * KC, D], BF16, tag="wfc")
            nc.sync.dma_start(w[:], wd.rearrange("(o p) f -> p o f", p=P))
            bt = const_pool.tile([1, D], BF16, tag=f"bfc_{nm}")
            nc.sync.dma_start(bt[:], bd.rearrange("(a d) -> a d", a=1))
            wfc_sb[nm] = w
            bfc_sb[nm] = bt

        def proj_feature_major(src_sb, w_sb, b_sb, out_sb, ofc, qh):
            # out^T[of, q] = sum_kc W[kc, of].T @ src^T[kc, q]  (+ bias row)
            ps = proj_ps_pool.tile([P, 512], F32, tag="proj")
            for kc in range(KC):
                nc.tensor.matmul(
                    ps[:],
                    lhsT=w_sb[:, kc, ofc * P:(ofc + 1) * P],
                    rhs=src_sb[:, kc, qh * 512:(qh + 1) * 512],
                    start=(kc == 0), stop=(kc == KC - 1),
                )
            nc.vector.tensor_scalar(
                out_sb[:, ofc, qh * 512:(qh + 1) * 512],
                ps[:],
                b_sb[:, ofc:ofc + 1],
                None,
                ALU.add,
            )

        def proj_token_major_vaug(src_sb, w_sb, bvb_sb, vaug_sb, tt):
            # V[t, f] token-major fp8, scattered into per-pair 192-col
            # regions: head A at cols 0:64, head B at cols 128:192.
            ps = proj_ps_pool.tile([P, 512], F32, tag="proj")
            for kc in range(KC):
                nc.tensor.matmul(
                    ps[:],
                    lhsT=src_sb[:, kc, tt * P:(tt + 1) * P],
                    rhs=w_sb[:, kc, :],
                    start=(kc == 0), stop=(kc == KC - 1),
                )
            psv = ps[:].rearrange("p (pr s c) -> p pr s c", s=2, c=DH)
            bvv = bvb_sb[:].rearrange("p (pr s c) -> p pr s c", s=2, c=DH)
            nc.vector.tensor_tensor(
                vaug_sb[:, tt, :, 0:DH],
                psv[:, :, 0, :], bvv[:, :, 0, :], ALU.add,
            )
            nc.vector.tensor_tensor(
                vaug_sb[:, tt, :, 2 * DH:3 * DH],
                psv[:, :, 1, :], bvv[:, :, 1, :], ALU.add,
            )

        attn_sb_of = {}   # branch name -> attn^T (incl Q residual), bf16

        def fc_tasks(out_name, br0, br1):
            # one closure per (ofc, qh) output tile of the fc
            out_dram = out_x_t if out_name == "x" else out_y_t
            res_sb = act_sb[out_name]

            def one(ofc, qh):
                w_sb = wfc_sb[out_name]
                b_sb = bfc_sb[out_name]
                srcs = [(attn_sb_of[br0], 0), (attn_sb_of[br1], KC)]
                ps = proj_ps_pool.tile([P, 512], F32, tag="proj")
                step = 0
                for src_sb, wbase in srcs:
                    for kc in range(KC):
                        nc.tensor.matmul(
                            ps[:],
                            lhsT=w_sb[:, wbase + kc, ofc * P:(ofc + 1) * P],
                            rhs=src_sb[:, kc, qh * 512:(qh + 1) * 512],
                            start=(step == 0), stop=False,
                        )
                        step += 1
                # fc bias as a rank-1 matmul (contract dim 1 of ones)
                nc.tensor.matmul(
                    ps[:],
                    lhsT=b_sb[0:1, ofc * P:(ofc + 1) * P],
                    rhs=ones_row[0:1, :],
                    start=False, stop=True,
                )
                # epilogue: relu(fc) + residual in one fused op; the
                # residual comes from the resident bf16 activations
                outt = stream_pool.tile([P, 512], F32, tag="outt")
                nc.vector.scalar_tensor_tensor(
                    outt[:], ps[:], 0.0,
                    res_sb[:, ofc, qh * 512:(qh + 1) * 512],
                    ALU.max, ALU.add)
                nc.sync.dma_start(
                    out_dram[ofc * P:(ofc + 1) * P, qh * 512:(qh + 1) * 512],
                    outt[:],
                )

            return [(lambda ofc=ofc, qh=qh: one(ofc, qh))
                    for ofc in range(KC) for qh in range(QH)]

        def fc_split(out_name, br0, br1, tiles, pool):
            # phase A: the br0-side contraction + bias (ready before
            # br1's normalize); returns phase-B completions
            out_dram = out_x_t if out_name == "x" else out_y_t
            res_sb = act_sb[out_name]
            w_sb = wfc_sb[out_name]
            b_sb = bfc_sb[out_name]
            done = []
            for ofc, qh in tiles:
                ps = pool.tile([P, 512], F32, tag="proj" if pool is proj_ps_pool else "av")
                src0 = attn_sb_of[br0]
                for kc in range(KC):
                    nc.tensor.matmul(
                        ps[:],
                        lhsT=w_sb[:, kc, ofc * P:(ofc + 1) * P],
                        rhs=src0[:, kc, qh * 512:(qh + 1) * 512],
                        start=(kc == 0), stop=False,
                    )
                nc.tensor.matmul(
                    ps[:],
                    lhsT=b_sb[0:1, ofc * P:(ofc + 1) * P],
                    rhs=ones_row[0:1, :],
                    start=False, stop=False,
                )

                def fin(ofc=ofc, qh=qh, ps=ps):
                    src1 = attn_sb_of[br1]
                    for kc in range(KC):
                        nc.tensor.matmul(
                            ps[:],
                            lhsT=w_sb[:, KC + kc, ofc * P:(ofc + 1) * P],
                            rhs=src1[:, kc, qh * 512:(qh + 1) * 512],
                            start=False, stop=(kc == KC - 1),
                        )
                    outt = stream_pool.tile([P, 512], F32, tag="outt")
                    nc.vector.scalar_tensor_tensor(
                        outt[:], ps[:], 0.0,
                        res_sb[:, ofc, qh * 512:(qh + 1) * 512],
                        ALU.max, ALU.add)
                    nc.sync.dma_start(
                        out_dram[ofc * P:(ofc + 1) * P,
                                 qh * 512:(qh + 1) * 512],
                        outt[:],
                    )
                done.append(fin)
            return done

        # ---- per-branch state + emission helpers -------------------------
        class Branch:
            pass

        def start_branch(bname, qsrc, kvsrc):
            """Emit weight DMAs + vaug memsets; return Branch with
            per-tile proj closures (to be interleaved as PE filler)."""
            br = Branch()
            br.name = bname
            br.quarters = (bname == "yy")
            w_sb = wsb_of[bname]
            bq_sb = w_sb["bq"]
            bk_sb = w_sb["bk"]
            bvb_sb = w_sb["bvb"]
            br.qt = q_pool.tile([P, KC, N], BF16, tag="qt")
            br.kt = kv_pool.tile([P, KC, N], BF16, tag="kt")
            br.vaug = kv_pool.tile([P, KT, NPAIR, VW], FP8, tag="vaug")
            # ones columns at region cols 64 and 127; zeros between
            nc.gpsimd.memset(br.vaug[:, :, :, DH:DH + 1], 1.0)
            nc.gpsimd.memset(br.vaug[:, :, :, DH + 1:2 * DH - 1], 0.0)
            nc.gpsimd.memset(br.vaug[:, :, :, 2 * DH - 1:2 * DH], 1.0)

            qs, kvs = act_sb[qsrc], act_sb[kvsrc]
            pre, early, rest = [], [], []
            for ofc in range(KC):
                for qh in range(QH):
                    # S of pair p reads BOTH K qh-halves of chunk p (the
                    # k axis spans all of N) but only its own Q half, so
                    # K(0,1) must precede the first S while Q(0,1) and V
                    # can ride in the first attention slot.
                    qdst = pre if (ofc == 0 and qh == 0) else (
                        early if ofc == 0 else rest)
                    kdst = pre if ofc == 0 else rest
                    qdst.append(lambda ofc=ofc, qh=qh: proj_feature_major(
                        qs, w_sb["q"], bq_sb, br.qt, ofc, qh))
                    kdst.append(lambda ofc=ofc, qh=qh: proj_feature_major(
                        kvs, w_sb["k"], bk_sb, br.kt, ofc, qh))
            for tt in range(KT):
                early.append(lambda tt=tt: proj_token_major_vaug(
                    kvs, w_sb["v"], bvb_sb, br.vaug, tt))
            br.pre_tasks = pre
            br.early_tasks = early
            br.rest_tasks = rest
            br.proj_tasks = pre + early + rest
            return br

        def emit_s_exp(br, pair, qh, n_dve=0, kts=None, e_sb=None):
            if e_sb is None:
                e_sb = e_pool.tile([P, KT, 2, 512], FP8, tag="e")
            for kt in (kts if kts is not None else range(KT)):
                ps = qk_ps_pool.tile([P, 1024], F32, tag="qk")
                # head A on array rows 0:64, head B on rows 64:128
                nc.tensor.matmul(
                    ps[:, 0:512],
                    lhsT=br.kt[0:DH, pair, kt * P:(kt + 1) * P],
                    rhs=br.qt[0:DH, pair, qh * 512:(qh + 1) * 512],
                    start=True, stop=True,
                )
                nc.tensor.matmul(
                    ps[:, 512:1024],
                    lhsT=br.kt[DH:P, pair, kt * P:(kt + 1) * P],
                    rhs=br.qt[DH:P, pair, qh * 512:(qh + 1) * 512],
                    start=True, stop=True,
                )
                if kt < n_dve:
                    # Polynomial exp off the ACT engine:
                    # e = (1+y+y^2/2)^2, y = x/2; |y| < ~0.35 so the
                    # truncation error is < ~1%, inside e's own fp8
                    # quantization noise.  P1 (the PSUM read) runs on
                    # DVE, the SBUF-only passes on the idle GPSIMD.
                    t_sb = poly_pool.tile([P, 1024], BF16, tag="pt")
                    nc.vector.tensor_scalar(
                        t_sb[:], ps[:], SCALE * 0.5, 1.0, ALU.mult, ALU.add)
                    u_sb = poly_pool.tile([P, 1024], BF16, tag="pu")
                    nc.vector.scalar_tensor_tensor(
                        u_sb[:], t_sb[:], 0.5, t_sb[:], ALU.mult, ALU.mult)
                    v_sb = poly_pool.tile([P, 1024], BF16, tag="pv")
                    nc.vector.tensor_scalar(
                        v_sb[:], u_sb[:], 0.5, None, ALU.add)
                    nc.vector.tensor_tensor(
                        e_sb[:, kt, :, :].rearrange("p a b -> p (a b)"),
                        v_sb[:], v_sb[:], ALU.mult)
                else:
                    nc.scalar.activation(
                        e_sb[:, kt, :, :], ps[:],
                        AF.Exp, scale=SCALE,
                    )
            return e_sb

        def pe_warm(pool, n):
            # tiny matmuls into a scratch PSUM tile: keep the PE busy
            # through otherwise-idle stretches so HAM stays at K=8/8
            # (a >3.4us PE lull re-throttles the clock to 1.2 GHz)
            tag = "proj" if pool is proj_ps_pool else "av"
            for _ in range(n):
                scr = pool.tile([P, 512], F32, tag=tag)
                nc.tensor.matmul(
                    scr[0:1, :], lhsT=ones_row[0:1, 0:1], rhs=ones_row[0:1, :],
                    start=True, stop=True)

        def dg_loc(br, pair, qh, hl):
            if br.quarters:
                return pair, 4 * (qh * 2 + hl)
            return pair // 2, 4 * ((pair % 2) * 4 + qh * 2 + hl)

        def emit_av(br, pair, qh, e_sb):
            for hl in range(2):
                ps_av = av_ps_pool.tile([P, 512], F32, tag="av")
                if USE_AV_DOUBLEROW:
                    # kt-pair (0,1) last: its E may come from the DVE
                    # polynomial, which finishes later than ACT exps
                    for t2 in (1, 2, 3, 0):
                        nc.tensor.matmul(
                            ps_av[:],
                            lhsT=br.vaug[:, 2 * t2:2 * t2 + 2, pair,
                                         hl * DH:hl * DH + P],
                            rhs=e_sb[:, 2 * t2:2 * t2 + 2, hl, :],
                            start=(t2 == 1), stop=(t2 == 0),
                            perf_mode=mybir.MatmulPerfMode.DoubleRow,
                        )
                else:
                    for kt in range(KT):
                        nc.tensor.matmul(
                            ps_av[:],
                            lhsT=br.vaug[:, kt, pair, hl * DH:hl * DH + P],
                            rhs=e_sb[:, kt, hl, :],
                            start=(kt == 0), stop=(kt == KT - 1),
                        )
                o2 = o2_pool.tile([P, 512], BF16, tag="o2")
                nc.vector.tensor_copy(o2[:], ps_av[:])
                br.o2_tiles[(pair, qh, hl)] = o2
                # scatter d row into the branch dg tile as [2, 256]
                drow = DH if hl == 0 else 0
                tidx, row = dg_loc(br, pair, qh, hl)
                nc.sync.dma_start(
                    br.dg[tidx][row:row + 4, :],
                    o2[drow:drow + 1, :],
                )

        def attention_branch(br, filler, late_filler=(), early_filler=(),
                             pend=None):
            """Emit the branch's attention with software pipelining:
            the AV of one group is emitted after the S+exp of the NEXT
            group (even across branch boundaries, via `pend`), and
            `filler` closures (next branch's projections, the previous
            branch's normalize, ready fc tiles) are interleaved to keep
            the PE fed during the ACT-bound exp stretches."""
            br.o2_tiles = {}
            attn_t = attn_pool.tile([P, KC, N], BF16, tag="attn")
            br.attn = attn_t
            attn_sb_of[br.name] = attn_t
            nt = 4 if br.quarters else 2
            rows = 64 // nt
            br.dg = []
            br.rr = []
            for h in range(nt):
                dg_t = dgr_pool.tile([rows, 128], BF16, tag=f"dg{h}")
                rr_t = dgr_pool.tile([rows, 128], BF16, tag=f"rr{h}")
                br.dg.append(dg_t)
                br.rr.append(rr_t)
            groups = [(pair, qh) for pair in range(NPAIR)
                      for qh in range(QH)][getattr(br, "skip_groups", 0):]
            nslots = len(groups) + 1
            fill_plan = [[] for _ in range(nslots)]
            for i, task in enumerate(filler):
                fill_plan[min(i * nslots // max(len(filler), 1), nslots - 1)].append(task)
            fill_plan[0][0:0] = list(early_filler)
            fill_plan[6].extend(late_filler)
            for g, (pair, qh) in enumerate(groups):
                e_sb = emit_s_exp(br, pair, qh, n_dve=0)
                if pend is not None:
                    emit_av(pend[0], *pend[1:])
                for task in fill_plan[g]:
                    task()
                pend = (br, pair, qh, e_sb)
            for task in fill_plan[len(groups)]:
                task()
            return pend

        def normalize_tile(br, tidx, pairs):
            # one wide reciprocal over the dg tile, then normalize its
            # pairs
            with nc.allow_low_precision("softmax denominators in bf16"):
                nc.vector.reciprocal(br.rr[tidx][:], br.dg[tidx][:])
            nrows = br.dg[tidx].shape[0]
            r_dram = dram_pool.tile([nrows, 128], BF16, tag="rd")
            nc.sync.dma_start(r_dram[:], br.rr[tidx][:])
            r_flat = r_dram.rearrange("a b -> (a b)")
            attn_sb = br.attn
            for pair in pairs:
                for qh in range(QH):
                    rbc = rbc_pool.tile([P, 512], BF16, tag="rbc")
                    tmp = ntmp_pool.tile([P, 512], BF16, tag="ntmp")
                    for hl in range(2):
                        rows = slice(hl * DH, (hl + 1) * DH)
                        hidx = dg_loc(br, pair, qh, hl)[1] // 4
                        nc.gpsimd.dma_start(
                            out=rbc[rows, :],
                            in_=r_flat[hidx * 512:(hidx + 1) * 512]
                            .partition_broadcast(DH),
                        )
                        o2 = br.o2_tiles[(pair, qh, hl)]
                        orows = slice(0, DH) if hl == 0 else slice(DH, P)
                        nc.vector.tensor_tensor(
                            tmp[rows, :], o2[orows, :], rbc[rows, :], ALU.mult)
                        nc.vector.tensor_tensor(
                            attn_sb[rows, pair, qh * 512:(qh + 1) * 512],
                            tmp[rows, :],
                            br.qt[rows, pair, qh * 512:(qh + 1) * 512],
                            ALU.add,
                        )

        def normalize_half(br, half):
            normalize_tile(br, half, (2 * half, 2 * half + 1))

        def normalize_quarter(br, pair):
            normalize_tile(br, pair, (pair,))

        def normalize_branch(br):
            normalize_half(br, 0)
            normalize_half(br, 1)

        # ---- pipelined branch schedule ----------------------------------
        # warm the ACT table (exp set) at t=0 with a dummy activation
        warm = const_pool.tile([1, 512], F32, tag="warm")
        nc.scalar.activation(warm[:], ones_row[:], AF.Exp, scale=1.0)

        br_xx = start_branch(*_BRANCHES[0])
        # handcrafted prologue: Q(0,0) + K(0,0) projections, then the
        # first 4 S k-tiles (which only read K columns 0:512), with
        # K(0,1) landing while their exps run
        for t in br_xx.pre_tasks:
            t()
        br_xy = start_branch(*_BRANCHES[1])
        pend = attention_branch(
            br_xx,
            [lambda: load_branch_weights("yx"),
             lambda: load_fc_weights("x")]
            + br_xx.rest_tasks + br_xy.proj_tasks,
            early_filler=br_xx.early_tasks)
        br_yx = start_branch(*_BRANCHES[2])
        pend = attention_branch(
            br_xy,
            [lambda: normalize_branch(br_xx),
             lambda: load_branch_weights("yy")]
            + br_yx.proj_tasks,
            pend=pend)
        fcx = fc_tasks("x", "xx", "xy")
        br_yy = start_branch(*_BRANCHES[3])
        pend = attention_branch(
            br_yx,
            [lambda: normalize_branch(br_xy)] + br_yy.proj_tasks + fcx[0:6],
            pend=pend)
        finish = []
        pend = attention_branch(
            br_yy,
            [lambda: normalize_branch(br_yx),
             lambda: load_fc_weights("y")],
            late_filler=[
                lambda: normalize_quarter(br_yy, 0),
                lambda: normalize_quarter(br_yy, 1),
                lambda: finish.extend(
                    fc_split("y", "yx", "yy", [(0, 0), (0, 1)],
                             proj_ps_pool)),
            ],
            early_filler=fcx[6:8],
            pend=pend)
        emit_av(pend[0], *pend[1:])
        normalize_quarter(br_yy, 2)
        fcy = fc_tasks("y", "yx", "yy")
        finish += fc_split("y", "yx", "yy", [(1, 0), (1, 1)], av_ps_pool)
        normalize_quarter(br_yy, 3)
        for f in finish:
            f()
        for t in fcy[4:8]:
            t()

    _split_excess_waits(nc)
    return nc


def _get_nc():
    global _CACHED_NC
    if _CACHED_NC is None:
        _CACHED_NC = _build_nc()
    return _CACHED_NC


def kernel(**inputs):
    global LAST_RESULT
    nc = _get_nc()

    X = np.asarray(inputs["X"], np.float32)
    Y = np.asarray(inputs["Y"], np.float32)

    def bf(a):
        return np.ascontiguousarray(a).astype(ml_dtypes.bfloat16)

    shared = {}
    for bn, _, _ in _BRANCHES:
        for t in ("q", "k", "v"):
            shared[f"w{t}_{bn}"] = bf(inputs[f"W_{t}_{bn}"])
            shared[f"b{t}_{bn}"] = np.asarray(inputs[f"b_{t}_{bn}"], np.float32)
    shared["wfc_x"] = bf(inputs["W_X"])
    shared["wfc_y"] = bf(inputs["W_Y"])
    shared["bfc_x"] = bf(inputs["b_X"])
    shared["bfc_y"] = bf(inputs["b_Y"])

    in_maps = []
    for b in range(B):
        xt = np.ascontiguousarray(X[b].T)
        yt = np.ascontiguousarray(Y[b].T)
        m = dict(shared)
        m["xt_bf"] = xt.astype(ml_dtypes.bfloat16)
        m["yt_bf"] = yt.astype(ml_dtypes.bfloat16)
        in_maps.append(m)

    res = run_bass_kernel_spmd(nc, in_maps, list(range(B)))
    LAST_RESULT = res

    out_x = np.stack([res.results[b]["out_x_t"].T for b in range(B)])
    out_y = np.stack([res.results[b]["out_y_t"].T for b in range(B)])
    return out_x.astype(np.float32), out_y.astype(np.float32)
